# revision 9
# baseline (speedup 1.0000x reference)
"""Trainium2 Bass kernel for AdvancedIntegratedFiberOpticsNN.

Sharding: 8 cores = 4 images x 2 H-halves (pure data parallel; the only
cross-half quantity, avg_g, is recomputed per core from the full image).

Per-core device program (one TileContext, ~2.4k instructions):
  phase 0: Sobel gradient of the full image in row-partition layout (DVE ops,
           vertical taps via DMA-shifted copies), sqrt+row-sum on ACT,
           partition-sum via a tiny matmul -> S_g -> per-channel scale s96,
           folded on-device into the pattern-conv and cls1 weights.
  phase 1: 3/5/7 input convs as 7 dx-major matmuls (K=21 row-stacked x, bf16)
           over 512-px flat chunks -> full half-image feature buffer (bf16).
  phase 2: per 512-px chunk: 3x3 pattern conv (9 taps, K=96 -> M=72),
           sigmoid, cls1 (K=96+72 -> 2x128), relu, cls2 (K=256 -> 128), relu,
           heads + anomaly-mean matmuls, output staged bf16, DMA-cast to f32.
"""
import os
import numpy as np
import ml_dtypes

import concourse.bass as bass
import concourse.mybir as mybir
import concourse.tile as tile
from concourse import bacc
from concourse.bass_utils import run_bass_kernel_spmd

F32 = mybir.dt.float32
BF16 = mybir.dt.bfloat16
AF = mybir.ActivationFunctionType
ALU = mybir.AluOpType
BF = ml_dtypes.bfloat16

B, H, W = 4, 256, 256
R = 128                  # output rows per core
WP = 264                 # padded width
COL0 = 3                 # image col 0 lives at padded col 3
FR = 130                 # feature rows per core (R + 2)
FLAT = FR * WP           # 34320
CHUNK = 512
N1 = 68                  # phase-1 chunks (covers [0, 34816))
N2 = 66                  # phase-2 chunks (covers [264, 34056))
G = 512                  # feature buffer guard (zeros) on each side
FEAT_LEN = G + N1 * CHUNK + G
XP_ROWS = 136
XP_LEN = 8 + (XP_ROWS + 4) * WP + 8
SGG = 4                  # chunks per stacked-x DMA group
NG = N1 // SGG
TAPS = [(dy, dx) for dy in (-1, 0, 1) for dx in (-1, 0, 1)]
OSH = 33 * CHUNK         # half of the output staging (64 rows)

_NC_CACHE = {}
LAST_RESULTS = None      # BassKernelResults of the most recent run (for test.py)


# --------------------------------------------------------------------------
# host-side preparation
# --------------------------------------------------------------------------

def _host_prep(inp):
    x = np.asarray(inp['x'], np.float32)
    w3, b3 = np.asarray(inp['w3'], np.float32), np.asarray(inp['b3'], np.float32)
    w5, b5 = np.asarray(inp['w5'], np.float32), np.asarray(inp['b5'], np.float32)
    w7, b7 = np.asarray(inp['w7'], np.float32), np.asarray(inp['b7'], np.float32)
    grad_w = np.asarray(inp['grad_w'], np.float32)
    pos_w = np.asarray(inp['pos_w'], np.float32)
    grad_adj = float(np.asarray(inp['grad_adj']))
    pos_adj = float(np.asarray(inp['pos_adj']))
    npat = np.asarray(inp['normal_patterns'], np.float32)
    thr = np.asarray(inp['normal_thresholds'], np.float32)
    apat = np.asarray(inp['anomaly_patterns'], np.float32)
    w1 = np.asarray(inp['cls_w1'], np.float32)[:, :, 0, 0]
    b1 = np.asarray(inp['cls_b1'], np.float32)
    w2 = np.asarray(inp['cls_w2'], np.float32)[:, :, 0, 0]
    b2 = np.asarray(inp['cls_b2'], np.float32)
    rw = np.asarray(inp['region_w'], np.float32)[:, :, 0, 0]
    rb = np.asarray(inp['region_b'], np.float32)
    aw = np.asarray(inp['anom_w'], np.float32)[:, :, 0, 0]
    ab = np.asarray(inp['anom_b'], np.float32)
    qw = np.asarray(inp['qual_w'], np.float32)[:, :, 0, 0]
    qb = np.asarray(inp['qual_b'], np.float32)

    ypos = np.linspace(-1.0, 1.0, H, dtype=np.float32).reshape(H, 1)
    xpos = np.linspace(-1.0, 1.0, W, dtype=np.float32).reshape(1, W)
    avg_p = float(np.sqrt(xpos ** 2 + ypos ** 2).mean())
    posf = 1.0 + pos_w * avg_p * pos_adj
    br_of = np.repeat(np.arange(3), 32)
    A_vec = posf[br_of].astype(np.float32).reshape(96, 1)
    Bv_vec = (posf * grad_w * grad_adj / (3.0 * H * W))[br_of]
    Bv_vec = Bv_vec.astype(np.float32).reshape(96, 1)

    wdx = np.zeros((21, 7, 96), np.float32)
    for co_base, wbr, k2 in ((0, w3, 1), (32, w5, 2), (64, w7, 3)):
        for r in range(7):
            dy = r - 3
            if abs(dy) > k2:
                continue
            for c in range(3):
                for dxi in range(7):
                    dx = dxi - 3
                    if abs(dx) > k2:
                        continue
                    wdx[r * 3 + c, dxi, co_base:co_base + 32] = \
                        wbr[:, c, dy + k2, dx + k2]
    fbias = np.concatenate([b3, b5, b7]).astype(np.float32).reshape(96, 1)

    pat = np.concatenate([npat, apat], axis=0)          # [72, 96, 3, 3]
    pw = np.zeros((96, 9, 72), np.float32)
    for dy in (-1, 0, 1):
        for dx in (-1, 0, 1):
            pw[:, (dy + 1) * 3 + (dx + 1), :] = pat[:, :, dy + 1, dx + 1].T

    w1f = np.ascontiguousarray(w1[:, 0:96].T)           # [96, 256]
    w1s = np.ascontiguousarray(w1[:, 96:168].T)         # [72, 256]
    w2k = np.concatenate([w2[:, 0:128].T, w2[:, 128:256].T], axis=1)  # [128,256]
    # head channels on-chip: partitions 0..2 = region, 32 = anom, 33 = qual
    # (quadrant-aligned so every engine op starts at partition 0 or 32;
    # remapped to output channel order by the out DMAs)
    wh = np.zeros((128, 34), np.float32)
    wh[:, 0:3] = rw.T
    wh[:, 32] = aw[0]
    wh[:, 33] = qw[0]
    bh = np.zeros((34, 1), np.float32)
    bh[0:3, 0] = rb
    bh[32, 0] = ab[0]
    bh[33, 0] = qb[0]
    b1v = np.stack([b1[0:128], b1[128:256]], axis=1).astype(np.float32)

    shared = {
        'wdx': wdx.astype(BF),
        'pw': pw.astype(BF).reshape(96, 9 * 72),
        'w1f': w1f.astype(BF),
        'w1s': w1s.astype(BF),
        'w2': w2k.astype(BF),
        'wh': wh.astype(BF),
        'on48': np.full((48, 1), -1.0 / 48.0, BF),
        'fbias': fbias,
        'thr': thr.reshape(48, 1).astype(np.float32),
        'b1': b1v,
        'b2': b2.reshape(128, 1).astype(np.float32),
        'bh': bh,
        'Av': A_vec,
        'Bv': Bv_vec,
    }

    cores = []
    for i in range(8):
        b, half = i // 2, i % 2
        r0 = R * half
        xp = np.zeros((3, XP_LEN), BF)
        body = np.zeros((3, XP_ROWS + 4, WP), np.float32)
        y0, y1 = max(0, r0 - 4), min(H, r0 - 4 + XP_ROWS)
        body[:, y0 - (r0 - 4):y1 - (r0 - 4), COL0:COL0 + W] = x[b, :, y0:y1, :]
        xp[:, 8:8 + (XP_ROWS + 4) * WP] = body.reshape(3, -1).astype(BF)
        xf = np.zeros((3, 2, 128, WP), np.float32)
        xf[:, :, :, COL0:COL0 + W] = x[b].reshape(3, 2, 128, W)
        msk = np.array([[1.0 if r0 > 0 else 0.0,
                         1.0 if r0 + R < H else 0.0]], np.float32)
        cores.append(dict(xp=xp, xf=xf, msk=msk, b=b, r0=r0))
    return shared, cores


# --------------------------------------------------------------------------
# device program
# --------------------------------------------------------------------------

def _build_nc():
    nc = bacc.Bacc(None, target_bir_lowering=False, debug=False)

    xp_t = nc.declare_dram_parameter("xp", [3, XP_LEN], BF16, isOutput=False)
    xf_t = nc.declare_dram_parameter("xf", [3, 2, 128, WP], F32, isOutput=False)
    msk_t = nc.declare_dram_parameter("msk", [1, 2], F32, isOutput=False)
    wdx_t = nc.declare_dram_parameter("wdx", [21, 7, 96], BF16, isOutput=False)
    pw_t = nc.declare_dram_parameter("pw", [96, 9 * 72], BF16, isOutput=False)
    w1f_t = nc.declare_dram_parameter("w1f", [96, 256], BF16, isOutput=False)
    w1s_t = nc.declare_dram_parameter("w1s", [72, 256], BF16, isOutput=False)
    w2_t = nc.declare_dram_parameter("w2", [128, 256], BF16, isOutput=False)
    wh_t = nc.declare_dram_parameter("wh", [128, 34], BF16, isOutput=False)
    on48_t = nc.declare_dram_parameter("on48", [48, 1], BF16, isOutput=False)
    fb_t = nc.declare_dram_parameter("fbias", [96, 1], F32, isOutput=False)
    thr_t = nc.declare_dram_parameter("thr", [48, 1], F32, isOutput=False)
    b1_t = nc.declare_dram_parameter("b1", [128, 2], F32, isOutput=False)
    b2_t = nc.declare_dram_parameter("b2", [128, 1], F32, isOutput=False)
    bh_t = nc.declare_dram_parameter("bh", [34, 1], F32, isOutput=False)
    av_t = nc.declare_dram_parameter("Av", [96, 1], F32, isOutput=False)
    bv_t = nc.declare_dram_parameter("Bv", [96, 1], F32, isOutput=False)
    out_t = nc.declare_dram_parameter("out", [5, R, W], F32, isOutput=True)

    with tile.TileContext(nc) as tc:
        import contextlib
        with contextlib.ExitStack() as ctx:
            consts = ctx.enter_context(tc.tile_pool(name="consts", bufs=1))
            big = ctx.enter_context(tc.tile_pool(name="big", bufs=1))
            sob = ctx.enter_context(tc.tile_pool(name="sob", bufs=1))
            sgp = ctx.enter_context(tc.tile_pool(name="sgp", bufs=4))
            work = ctx.enter_context(tc.tile_pool(name="work", bufs=2))
            sap = ctx.enter_context(tc.tile_pool(name="sap", bufs=3))
            osp = ctx.enter_context(tc.tile_pool(name="osp", bufs=1))

            # ---- constants into SBUF ----
            wdx_sb = consts.tile([21, 7, 96], BF16)
            nc.sync.dma_start(out=wdx_sb, in_=wdx_t[:, :, :])
            pw_sb = consts.tile([96, 9 * 72], BF16)
            nc.sync.dma_start(out=pw_sb, in_=pw_t[:, :])
            w1f_sb = consts.tile([96, 256], BF16)
            nc.sync.dma_start(out=w1f_sb, in_=w1f_t[:, :])
            w1s_sb = consts.tile([72, 256], BF16)
            nc.sync.dma_start(out=w1s_sb, in_=w1s_t[:, :])
            w2_sb = consts.tile([128, 256], BF16)
            nc.sync.dma_start(out=w2_sb, in_=w2_t[:, :])
            wh_sb = consts.tile([128, 34], BF16)
            nc.sync.dma_start(out=wh_sb, in_=wh_t[:, :])
            on48_sb = consts.tile([48, 1], BF16)
            nc.sync.dma_start(out=on48_sb, in_=on48_t[:, :])
            fb_sb = consts.tile([96, 1], F32)
            nc.sync.dma_start(out=fb_sb, in_=fb_t[:, :])
            thr_sb = consts.tile([48, 1], F32)
            nc.sync.dma_start(out=thr_sb, in_=thr_t[:, :])
            b1_sb = consts.tile([128, 2], F32)
            nc.sync.dma_start(out=b1_sb, in_=b1_t[:, :])
            b2_sb = consts.tile([128, 1], F32)
            nc.sync.dma_start(out=b2_sb, in_=b2_t[:, :])
            bh_sb = consts.tile([34, 1], F32)
            nc.sync.dma_start(out=bh_sb, in_=bh_t[:, :])
            av_sb = consts.tile([96, 1], F32)
            nc.sync.dma_start(out=av_sb, in_=av_t[:, :])
            bv_sb = consts.tile([96, 1], F32)
            nc.sync.dma_start(out=bv_sb, in_=bv_t[:, :])
            msk_sb = consts.tile([1, 2], F32)
            nc.sync.dma_start(out=msk_sb, in_=msk_t[:, :])
            ones96 = consts.tile([1, 96], F32)
            nc.vector.memset(ones96, 1.0)
            ones128 = consts.tile([128, 1], F32)
            nc.vector.memset(ones128, 1.0)
            pwf_sb = consts.tile([96, 9 * 72], BF16)   # folded pattern weights
            w1ff_sb = consts.tile([96, 256], BF16)     # folded cls1 feat weights
            m96 = consts.tile([96, 2], F32)            # row masks broadcast
            s96 = consts.tile([96, 1], F32)            # feature scale vector
            sgsc = consts.tile([1, 1], F32)            # S_g scalar

            # ---- big buffers ----
            features = big.tile([96, FEAT_LEN], BF16)

            with tc.tile_pool(name="ps0", bufs=1, space="PSUM") as ps0, \
                 tc.tile_pool(name="pfp", bufs=2, space="PSUM") as pfp:

                # ================= phase 0: Sobel / S_g =================
                xr = sob.tile([128, 2, 3, WP], F32)
                for c in range(3):
                    nc.sync.dma_start(
                        out=xr[:, :, c, :],
                        in_=xf_t[c, :, :, :].rearrange("b p w -> p b w"))
                gray = sob.tile([128, 2, WP], F32)
                nc.vector.tensor_add(gray, xr[:, :, 0, :], xr[:, :, 1, :])
                nc.vector.tensor_add(gray, gray, xr[:, :, 2, :])
                cd = sob.tile([128, 2, 256], F32)
                nc.vector.tensor_sub(cd, gray[:, :, 4:260], gray[:, :, 2:258])
                St = sob.tile([128, 2, 256], F32)
                q2 = sob.tile([128, 2, 256], F32)
                nc.vector.tensor_add(St, gray[:, :, 2:258], gray[:, :, 4:260])
                nc.vector.tensor_scalar_mul(q2, gray[:, :, 3:259], 2.0)
                nc.vector.tensor_add(St, St, q2)
                # vertical shifts via DMA copies (partition-safe)
                zrow = sob.tile([1, 256], F32)
                nc.vector.memset(zrow, 0.0)
                Su = sob.tile([128, 2, 256], F32)      # Su[r] = S[r+1]
                Sd = sob.tile([128, 2, 256], F32)      # Sd[r] = S[r-1]
                nc.sync.dma_start(out=Su[0:127, :, :], in_=St[1:128, :, :])
                nc.sync.dma_start(out=Su[127:128, 0, :], in_=St[0:1, 1, :])
                nc.sync.dma_start(out=Su[127:128, 1, :], in_=zrow)
                nc.sync.dma_start(out=Sd[1:128, :, :], in_=St[0:127, :, :])
                nc.sync.dma_start(out=Sd[0:1, 1, :], in_=St[127:128, 0, :])
                nc.vector.memset(Sd[0:1, 0, :], 0.0)
                cu = sob.tile([128, 2, 256], F32)      # cu[r] = cd[r+1]
                cdn = sob.tile([128, 2, 256], F32)     # cdn[r] = cd[r-1]
                nc.sync.dma_start(out=cu[0:127, :, :], in_=cd[1:128, :, :])
                nc.sync.dma_start(out=cu[127:128, 0, :], in_=cd[0:1, 1, :])
                nc.sync.dma_start(out=cu[127:128, 1, :], in_=zrow)
                nc.sync.dma_start(out=cdn[1:128, :, :], in_=cd[0:127, :, :])
                nc.sync.dma_start(out=cdn[0:1, 1, :], in_=cd[127:128, 0, :])
                nc.vector.memset(cdn[0:1, 0, :], 0.0)
                gy = sob.tile([128, 2, 256], F32)
                nc.vector.tensor_sub(gy, Su, Sd)
                gx = sob.tile([128, 2, 256], F32)
                nc.vector.tensor_scalar_mul(gx, cd, 2.0)
                nc.vector.tensor_add(gx, gx, cu)
                nc.vector.tensor_add(gx, gx, cdn)
                nc.vector.tensor_mul(gx, gx, gx)
                nc.vector.tensor_mul(gy, gy, gy)
                nc.vector.tensor_add(gx, gx, gy)
                gsc = sob.tile([128, 2, 256], F32)
                rs = sob.tile([128, 1], F32)
                nc.scalar.activation(gsc, gx, AF.Sqrt, accum_out=rs)
                # S_g = sum over partitions of rs
                sg_ps = ps0.tile([1, 1], F32, tag="sg")
                nc.tensor.matmul(sg_ps, lhsT=rs, rhs=ones128, start=True, stop=True)
                nc.vector.tensor_copy(sgsc, sg_ps)
                # broadcast S_g to 96 partitions via matmul with ones
                bc_ps = ps0.tile([96, 1], F32, tag="bc")
                nc.tensor.matmul(bc_ps, lhsT=ones96, rhs=sgsc, start=True, stop=True)
                nc.vector.tensor_scalar(s96, bc_ps, bv_sb, av_sb,
                                        op0=ALU.mult, op1=ALU.add)
                # fold scale into pattern + cls1-feature weights
                nc.vector.tensor_scalar_mul(pwf_sb, pw_sb, s96)
                nc.vector.tensor_scalar_mul(w1ff_sb, w1f_sb, s96)
                # broadcast row masks to 96 partitions
                mk_ps = ps0.tile([96, 2], F32, tag="mk")
                nc.tensor.matmul(mk_ps, lhsT=ones96, rhs=msk_sb, start=True, stop=True)
                nc.vector.tensor_copy(m96, mk_ps)

                # ================= phase 1: input convs =================
                nc.vector.memset(features[:, 0:G], 0.0)
                nc.vector.memset(features[:, FEAT_LEN - G:FEAT_LEN], 0.0)
                for g in range(NG):
                    sc = sgp.tile([21, SGG * CHUNK + 8], BF16, name=f"sc{g}",
                                  tag="sc")
                    in_ap = bass.AP(
                        tensor=xp_t[:, :].tensor,
                        offset=8 + g * SGG * CHUNK - 3,
                        ap=[[WP, 7], [XP_LEN, 3], [1, SGG * CHUNK + 6]])
                    nc.sync.dma_start(out=sc[:, 0:SGG * CHUNK + 6], in_=in_ap)
                    for qq in range(SGG):
                        s = (g * SGG + qq) * CHUNK
                        pf = pfp.tile([96, CHUNK], F32, name=f"pf{g}_{qq}",
                                      tag="pf")
                        for dxi in range(7):
                            nc.tensor.matmul(
                                pf, lhsT=wdx_sb[:, dxi, :],
                                rhs=sc[:, qq * CHUNK + dxi:
                                       qq * CHUNK + dxi + CHUNK],
                                start=(dxi == 0), stop=(dxi == 6))
                        nc.scalar.activation(
                            out=features[:, G + s:G + s + CHUNK], in_=pf,
                            func=AF.Identity, bias=fb_sb, scale=1.0)

                # boundary-row masks + zero the pattern-conv pad columns
                nc.vector.tensor_scalar_mul(
                    features[:, G:G + WP], features[:, G:G + WP], m96[:, 0:1])
                r129 = G + 129 * WP
                nc.vector.tensor_scalar_mul(
                    features[:, r129:r129 + WP], features[:, r129:r129 + WP],
                    m96[:, 1:2])
                fv = features[:, G:G + FLAT].rearrange("p (r w) -> p r w", w=WP)
                nc.vector.memset(fv[:, :, 2:3], 0.0)
                nc.vector.memset(fv[:, :, 259:260], 0.0)

            # ================= phase 2: pattern + classifier =================
            with tc.tile_pool(name="ppp", bufs=2, space="PSUM") as ppp, \
                 tc.tile_pool(name="ps1p", bufs=1, space="PSUM") as ps1p, \
                 tc.tile_pool(name="ps2p", bufs=2, space="PSUM") as ps2p, \
                 tc.tile_pool(name="php", bufs=1, space="PSUM") as php, \
                 tc.tile_pool(name="pmp", bufs=1, space="PSUM") as pmp:
                for hh in range(2):
                    os_t = osp.tile([34, OSH], BF16, name=f"os{hh}", tag="os")
                    for jl in range(33):
                        j = hh * 33 + jl
                        t = WP + j * CHUNK
                        base = G + t
                        # pattern conv: 9 taps accumulate
                        pp = ppp.tile([72, CHUNK], F32, name=f"pp{j}", tag="pp")
                        for ti, (dy, dx) in enumerate(TAPS):
                            o = dy * WP + dx
                            nc.tensor.matmul(
                                pp, lhsT=pwf_sb.rearrange(
                                    "p (t m) -> p t m", t=9)[:, ti, :],
                                rhs=features[:, base + o:base + o + CHUNK],
                                start=(ti == 0), stop=(ti == 8))
                        sa = sap.tile([72, CHUNK], BF16, name=f"sa{j}", tag="sa")
                        nc.scalar.activation(sa, pp, AF.Sigmoid)
                        an = sap.tile([48, CHUNK], BF16, name=f"an{j}", tag="an")
                        nc.vector.tensor_scalar(an, sa[0:48, :], thr_sb, 0.0,
                                                op0=ALU.subtract, op1=ALU.min)
                        # cls1: two M-chunks, K = 96 (features) + 72 (sa)
                        ps1 = ps1p.tile([128, 2 * CHUNK], F32, name=f"ps1_{j}",
                                        tag="ps1")
                        for m in range(2):
                            nc.tensor.matmul(
                                ps1[:, m * CHUNK:(m + 1) * CHUNK],
                                lhsT=w1ff_sb[:, m * 128:(m + 1) * 128],
                                rhs=features[:, base:base + CHUNK],
                                start=True, stop=False)
                            nc.tensor.matmul(
                                ps1[:, m * CHUNK:(m + 1) * CHUNK],
                                lhsT=w1s_sb[:, m * 128:(m + 1) * 128],
                                rhs=sa, start=False, stop=True)
                        h = work.tile([128, 2 * CHUNK], BF16, name=f"h{j}",
                                      tag="h")
                        nc.scalar.activation(h[:, 0:CHUNK], ps1[:, 0:CHUNK],
                                             AF.Relu, bias=b1_sb[:, 0:1])
                        nc.vector.tensor_scalar(h[:, CHUNK:], ps1[:, CHUNK:],
                                                b1_sb[:, 1:2], 0.0,
                                                op0=ALU.add, op1=ALU.max)
                        # cls2
                        ps2 = ps2p.tile([128, CHUNK], F32, name=f"ps2_{j}",
                                        tag="ps2")
                        nc.tensor.matmul(ps2, lhsT=w2_sb[:, 0:128],
                                         rhs=h[:, 0:CHUNK], start=True,
                                         stop=False)
                        nc.tensor.matmul(ps2, lhsT=w2_sb[:, 128:256],
                                         rhs=h[:, CHUNK:], start=False,
                                         stop=True)
                        p = work.tile([128, CHUNK], BF16, name=f"p{j}", tag="p")
                        nc.scalar.activation(p, ps2, AF.Relu, bias=b2_sb)
                        # heads (partitions 0..2 region, 32 anom, 33 qual)
                        ph = php.tile([34, CHUNK], F32, name=f"ph{j}", tag="ph")
                        nc.tensor.matmul(ph, lhsT=wh_sb, rhs=p, start=True,
                                         stop=True)
                        pm = pmp.tile([1, CHUNK], F32, name=f"pm{j}", tag="pm")
                        nc.tensor.matmul(pm, lhsT=on48_sb, rhs=an, start=True,
                                         stop=True)
                        osl = os_t[:, jl * CHUNK:(jl + 1) * CHUNK]
                        nc.scalar.activation(osl[32:34, :], ph[32:34, :],
                                             AF.Sigmoid, bias=bh_sb[32:34, :])
                        nc.vector.tensor_add(osl[32:33, :], osl[32:33, :], pm)
                        nc.vector.tensor_scalar_add(osl[0:3, :], ph[0:3, :],
                                                    bh_sb[0:3, :])
                    # DMA out this half (bf16 -> f32 cast => SWDGE)
                    ov = os_t.rearrange("p (r w) -> p r w", w=WP)
                    nc.gpsimd.dma_start(
                        out=out_t[0:3, hh * 64:(hh + 1) * 64, :],
                        in_=ov[0:3, :, COL0:COL0 + W])
                    nc.gpsimd.dma_start(
                        out=out_t[3:5, hh * 64:(hh + 1) * 64, :],
                        in_=ov[32:34, :, COL0:COL0 + W])
    nc.compile()
    return nc


def _get_nc():
    if 'nc' not in _NC_CACHE:
        _NC_CACHE['nc'] = _build_nc()
    return _NC_CACHE['nc']


# --------------------------------------------------------------------------
# entry point
# --------------------------------------------------------------------------

def kernel(**inputs) -> np.ndarray:
    global LAST_RESULTS
    nc = _get_nc()
    shared, cores = _host_prep(inputs)
    in_maps = []
    for ci in cores:
        m = dict(shared)
        m['xp'] = ci['xp']
        m['xf'] = ci['xf']
        m['msk'] = ci['msk']
        in_maps.append(m)
    res = run_bass_kernel_spmd(nc, in_maps, core_ids=list(range(8)),
                               trace=bool(os.environ.get("BASS_TRACE")))
    LAST_RESULTS = res
    full = np.zeros((B, 5, H, W), np.float32)
    for i, ci in enumerate(cores):
        full[ci['b'], :, ci['r0']:ci['r0'] + R, :] = res.results[i]['out']
    return full
